# revision 41
# baseline (speedup 1.0000x reference)
"""Trainium2 Bass kernel for BehlerG2-style symmetry functions.

Math (per (b,n,t) triple):
    s    = r_ij^2 + r_ik^2 + r_jk^2
    cut  = fc(r_ij)*fc(r_ik)*fc(r_jk),  fc(r) = 0.5*(cos(pi*r/6)+1)
    u    = 1 - s / (2*r_ij*r_ik)                  # = 1 - cos_theta
    W_e  = exp(-eta_e * s)                        # e in [0,16)
    v_p  = cut*mask * u^zeta_p                    # zeta = [1,2,4,8]
    f[b,n,e,z'] = sum_t W_e * v_p(z') * c_z'      # c = 2^(1-+zeta)

Mapping (per core, 128 atoms x 512 triples, SPMD over 8 cores):
  - elementwise in [atom-partition, triple-free] layout
  - PE-transpose s, u, cut*mask*u into [triple-partition, atom-free]
  - ACT: one Sin instr (3 cutoff cosines), 16 Exp instrs (scale=-eta baked)
  - PE: block-diagonal batched matmul, stationary V [t,(x,p)], moving
    W [t,(x,e)], PSUM-accumulated over 4 triple-chunks
  - diagonal extraction via DRAM round-trip gather, scale by 2^(1+-zeta)
"""

import math
import sys

import numpy as np

sys.path.insert(0, "/opt/trn_rl_repo")

_PROG_CACHE = {}

B, N, T = 4, 256, 512
E, Z = 16, 4
NCORES = 8
XA = (B * N) // NCORES  # atoms per core = 128
NG = 4  # atom groups per core (32 atoms each)
GSZ = XA // NG  # 32
NC_ = 4  # triple chunks (T/128)


def _np_reference(r_ij, r_ik, r_jk, mask_triples, etas, zetas):
    """Exact numpy fallback (matches reference.py) for unexpected params."""
    RC = 6.0

    def cut_fn(r):
        return np.where(r < RC, 0.5 * (np.cos(np.pi * r / RC) + 1.0), 0.0)

    r2 = r_ij**2 + r_ik**2 + r_jk**2
    cut = cut_fn(r_ij) * cut_fn(r_ik) * cut_fn(r_jk)
    radius = np.exp(-r2[..., None] * etas) * cut[..., None]
    cos_t = r2 / (2.0 * r_ij * r_ik)
    cos_t = np.where(mask_triples == 0, 0.0, cos_t)
    base = (1.0 - cos_t)[..., None] ** zetas
    ang = np.concatenate(
        [2.0 ** (1.0 - zetas) * base, 2.0 ** (1.0 + zetas) * base], axis=-1
    )
    f = np.einsum("bnt,bnte,bntz->bnez", mask_triples, radius, ang)
    return f.reshape(B, N, -1).astype(np.float32)


def build_core_kernel(tc, out_ap, in_aps, etas, zetas):
    """Emit one core's program into TileContext tc.

    in_aps: dict name -> DRAM AP [128, 512] f32
    out_ap: DRAM AP [128, 128] f32
    """
    from contextlib import ExitStack

    import concourse.bass as bass
    import concourse.mybir as mybir
    from concourse import masks

    nc = tc.nc
    f32 = mybir.dt.float32
    Alu = mybir.AluOpType
    Act = mybir.ActivationFunctionType

    bf16 = mybir.dt.bfloat16
    N_WARM = 12  # junk matmuls that keep the PE clock un-throttled

    ctx = ExitStack()
    pool = ctx.enter_context(tc.tile_pool(name="main", bufs=1))
    psum = ctx.enter_context(tc.tile_pool(name="psum", bufs=1, space="PSUM"))
    dram = ctx.enter_context(tc.tile_pool(name="dram", bufs=1, space="DRAM"))
    scratch = dram.tile([NG, 128, 512], f32)

    # ---- tiles (x-part layout: [atom-partition, ...triple-free]) ----
    rstack = pool.tile([128, 3, T], f32)  # r_ij | r_ik | r_jk
    mask_n = pool.tile([128, T], f32)
    cstack = pool.tile([128, 3, T], f32)  # cos(pi r / 6)
    cutst = pool.tile([128, 3, T], f32)  # 0.5 cos + 0.5
    sq1 = pool.tile([128, T], f32)
    sq2 = pool.tile([128, T], f32)
    sq3 = pool.tile([128, T], f32)
    s01 = pool.tile([128, T], f32)
    s_x = pool.tile([128, T], f32)
    prod = pool.tile([128, T], f32)
    rec = pool.tile([128, T], f32)
    m1 = pool.tile([128, T], f32)
    u_x = pool.tile([128, T], f32)
    cut12 = pool.tile([128, T], f32)
    cut3 = pool.tile([128, T], f32)
    cm = pool.tile([128, T], f32)
    w1_x = pool.tile([128, T], f32)
    ident = pool.tile([128, 128], f32)

    # t-part layout tiles: [triple-partition, chunk, atom]
    u_T = pool.tile([128, NC_, 128], f32)
    u2_T = pool.tile([128, NC_, 128], f32)
    u4_T = pool.tile([128, NC_, 128], f32)
    w1_T = pool.tile([128, NC_, 128], f32)
    v2f = pool.tile([128, NC_, 128], f32)
    v4f = pool.tile([128, NC_, 128], f32)
    v8f = pool.tile([128, NC_, 128], f32)
    # stationary (V) needs a single-free-dim AP -> (atom, p) contiguous minor.
    # moving (W) tolerates a 2D strided AP -> store e-major so the 16 exp
    # activations write contiguous slices (strided ACT writes are 2-4x slower)
    f32r = mybir.dt.float32r
    V = pool.tile([128, NC_, XA, Z], f32r)  # (chunk, atom, p)
    W = pool.tile([128, E, NC_, 128], f32r)  # (e, chunk, atom)
    jl = pool.tile([128, 128], bf16)
    jr = pool.tile([128, 512], bf16)

    Gs = pool.tile([128, NG, 512], f32)  # PSUM drain staging
    D = pool.tile([128, Z, E], f32)  # gathered diagonal
    f_t = pool.tile([128, E, 2 * Z], f32)  # final output tile

    psS = psum.tile([128, NC_, 128], f32)
    psU = psum.tile([128, NC_, 128], f32)
    psW1 = psum.tile([128, NC_, 128], f32)
    psG = [psum.tile([128, 512], f32, name=f"psG{g}") for g in range(NG)]
    psJ = psum.tile([128, 512], f32)

    # ---- loads: halves over the three DMA-capable queues ----
    H = T // 2
    nc.sync.dma_start(rstack[:, 0, 0:H], in_aps["r_ij"][:, 0:H])
    nc.sync.dma_start(rstack[:, 2, 0:H], in_aps["r_jk"][:, 0:H])
    nc.sync.dma_start(rstack[:, 0, H:T], in_aps["r_ij"][:, H:T])
    nc.sync.dma_start(rstack[:, 2, H:T], in_aps["r_jk"][:, H:T])
    nc.scalar.dma_start(rstack[:, 1, 0:H], in_aps["r_ik"][:, 0:H])
    nc.scalar.dma_start(rstack[:, 1, H:T], in_aps["r_ik"][:, H:T])
    nc.sync.dma_start(mask_n[:], in_aps["mask_triples"][:])
    masks.make_identity(nc, ident[:])
    neg_half_pi = pool.tile([128, 1], f32)
    nc.gpsimd.memset(neg_half_pi[:], -math.pi / 2.0)
    nc.gpsimd.memset(jl[:], 0.5)
    nc.gpsimd.memset(jr[:], 0.5)
    dsin = pool.tile([128, 1], f32)
    nc.gpsimd.memset(dsin[:], 0.0)

    # ---- ACT: cutoff cosines (one instr), later 16 exps ----
    # cos(pi r/6) = -sin(pi/6 * r - pi/2); ACT Sin needs args in [-pi, pi]
    # dummy sin first: triggers the trig table load before the input DMAs land
    nc.scalar.activation(dsin[:], dsin[:], Act.Sin, bias=neg_half_pi[:])
    for i in range(3):
        nc.scalar.activation(
            cstack[:, i, :], rstack[:, i, :], Act.Sin,
            bias=neg_half_pi[:], scale=math.pi / 6.0,
        )

    # ---- DVE: squares in halves (gates the exps via sq transposes) ----
    for h in range(2):
        sl = slice(h * H, (h + 1) * H)
        nc.vector.tensor_mul(sq1[:, sl], rstack[:, 0, sl], rstack[:, 0, sl])
        nc.vector.tensor_mul(sq2[:, sl], rstack[:, 1, sl], rstack[:, 1, sl])
        nc.vector.tensor_mul(sq3[:, sl], rstack[:, 2, sl], rstack[:, 2, sl])

    # ---- PE: s in t-layout via accumulating transposes of the squares ----
    for c in range(NC_):
        cs = slice(c * 128, (c + 1) * 128)
        for j, sq in enumerate((sq1, sq2, sq3)):
            nc.tensor.matmul(
                psS[:, c, :], sq[:, cs], ident[:],
                is_transpose=True, start=(j == 0), stop=(j == 2),
            )

    # ---- DVE: s in x-layout for the u chain ----
    nc.vector.tensor_add(s01[:], sq1[:], sq2[:])
    nc.vector.tensor_add(s_x[:], s01[:], sq3[:])

    # ---- DVE: u chain ----
    nc.vector.tensor_mul(prod[:], rstack[:, 0, :], rstack[:, 1, :])
    nc.vector.reciprocal_approx_fast(rec[:], prod[:])
    nc.vector.tensor_mul(m1[:], s_x[:], rec[:])
    nc.vector.tensor_scalar(u_x[:], m1[:], -0.5, 1.0, Alu.mult, Alu.add)

    # ---- cutoff product chain (tree: gpsimd and DVE halves in parallel) ----
    nc.gpsimd.tensor_scalar(cutst[:], cstack[:], -0.5, 0.5, Alu.mult, Alu.add)
    nc.gpsimd.tensor_mul(cut12[:], cutst[:, 0, :], cutst[:, 1, :])
    nc.vector.tensor_mul(cut3[:], cutst[:, 2, :], mask_n[:])
    nc.vector.tensor_mul(cm[:], cut3[:], u_x[:])
    nc.vector.tensor_mul(w1_x[:], cut12[:], cm[:])

    # ---- ACT: 16 exps, reading transposed s straight from PSUM ----
    for e in range(E):
        nc.scalar.activation(W[:, e], psS[:], Act.Exp, scale=-float(etas[e]))

    # ---- PE: transpose u and w1 ----
    for c in range(NC_):
        nc.tensor.transpose(psU[:, c, :], u_x[:, c * 128 : (c + 1) * 128], ident[:])
    for c in range(NC_):
        nc.tensor.transpose(psW1[:, c, :], w1_x[:, c * 128 : (c + 1) * 128], ident[:])

    # ---- DVE: drain transposes, build V (fp32 ladder + rounded f32r writes) ----
    nc.vector.tensor_copy(u_T[:], psU[:])
    nc.vector.tensor_mul(u2_T[:], u_T[:], u_T[:])
    nc.vector.tensor_mul(u4_T[:], u2_T[:], u2_T[:])
    nc.vector.tensor_mul(v2f[:], psW1[:], u_T[:])  # w1 * u
    nc.vector.tensor_mul(v4f[:], v2f[:], u2_T[:])  # w1 * u^3
    nc.vector.tensor_copy(V[:, :, :, 0], psW1[:])
    nc.vector.tensor_copy(V[:, :, :, 1], v2f[:])
    nc.vector.tensor_mul(V[:, :, :, 2], v2f[:], u2_T[:])
    nc.vector.tensor_mul(V[:, :, :, 3], v4f[:], u4_T[:])

    # ---- PE: junk matmuls to hold the HAM clock gate open through the
    # exp phase (PE otherwise idles >3.4us and drops to 1.2 GHz) ----
    for k in range(N_WARM):
        nc.tensor.matmul(psJ[:], jl[:], jr[:], start=(k == 0), stop=(k == N_WARM - 1))
    # exp-staggered junk: junk e reads W[:, e] so it becomes runnable only
    # after exp e completes -- fills the 25-31us PE idle that otherwise
    # drops the HAM clock to half speed for the real matmuls
    for e in range(0, 16):
        nc.tensor.matmul(
            psJ[:], W[:, e, 0, :].opt(), W[:, e].opt(), start=True, stop=True
        )

    # ---- PE: block-diagonal batched matmul ----
    # lhsT [t, (x,p)] stationary, rhs [t, (x,e)] moving, accum over chunks
    for g in range(NG):
        for c in range(NC_):
            lhsT = V[:, c, g * GSZ : (g + 1) * GSZ, :].opt()
            rhs = W[:, :, c, g * GSZ : (g + 1) * GSZ].transpose([0, 2, 1])
            nc.tensor.matmul(
                psG[g][:], lhsT, rhs, start=(c == 0), stop=(c == NC_ - 1)
            )
        # drain + DRAM round trip for diagonal gather
        nc.vector.tensor_copy(Gs[:, g, :], psG[g][:])
        sc_g = scratch[g]
        (nc.gpsimd if g % 2 == 0 else nc.sync).dma_start(sc_g, Gs[:, g, :])
        src = bass.AP(
            sc_g.tensor,
            sc_g.offset,
            [[4 * 512 + 16, GSZ], [512, Z], [1, E]],
        )
        (nc.sync if g % 2 == 0 else nc.scalar).dma_start(
            D[g * GSZ : (g + 1) * GSZ, :, :], src
        )

    # ---- scale into final layout: f[x, e, z'] ----
    for zc in range(2 * Z):
        zi = zc % Z
        if zc < Z:
            sc = 2.0 ** (1.0 - float(zetas[zi]))
        else:
            sc = 2.0 ** (1.0 + float(zetas[zi]))
        nc.vector.tensor_scalar(f_t[:, :, zc], D[:, zi, :], sc, None, Alu.mult)

    nc.sync.dma_start(out_ap[:], f_t[:])
    ctx.close()


def _build_program(etas, zetas):
    import concourse.bacc as bacc
    import concourse.mybir as mybir
    import concourse.tile as tile

    f32 = mybir.dt.float32
    nc = bacc.Bacc("TRN2", target_bir_lowering=False, debug=False, num_devices=NCORES)

    in_aps = {}
    for name in ("r_ij", "r_ik", "r_jk", "mask_triples"):
        in_aps[name] = nc.declare_dram_parameter(name, [XA, T], f32, isOutput=False).ap()
    out_ap = nc.declare_dram_parameter("out", [XA, E * 2 * Z], f32, isOutput=True).ap()

    with tile.TileContext(nc) as tc:
        build_core_kernel(tc, out_ap, in_aps, etas, zetas)
    nc.compile()
    return nc


def _get_program(etas, zetas):
    key = (tuple(float(x) for x in etas), tuple(float(x) for x in zetas))
    if key not in _PROG_CACHE:
        _PROG_CACHE[key] = _build_program(etas, zetas)
    return _PROG_CACHE[key]


def kernel(r_ij, r_ik, r_jk, mask_triples, etas, zetas):
    etas = np.asarray(etas, np.float32)
    zetas = np.asarray(zetas, np.float32)
    args = dict(r_ij=r_ij, r_ik=r_ik, r_jk=r_jk, mask_triples=mask_triples)

    # fast path requires zeta = [1, 2, 4, 8] (powers computed by squaring)
    if (
        tuple(zetas.tolist()) != (1.0, 2.0, 4.0, 8.0)
        or r_ij.shape != (B, N, T)
        or float(np.max(np.abs([r_ij.max(), r_ik.max(), r_jk.max()]))) >= 6.0
    ):
        return _np_reference(
            np.asarray(r_ij), np.asarray(r_ik), np.asarray(r_jk),
            np.asarray(mask_triples), etas, zetas,
        )

    from concourse.bass_utils import run_bass_kernel_spmd

    nc = _get_program(etas, zetas)
    flat = {k: np.ascontiguousarray(np.asarray(v, np.float32).reshape(B * N, T))
            for k, v in args.items()}
    in_maps = [
        {k: v[c * XA : (c + 1) * XA] for k, v in flat.items()} for c in range(NCORES)
    ]
    res = run_bass_kernel_spmd(nc, in_maps, list(range(NCORES)))
    out = np.concatenate([res.results[c]["out"] for c in range(NCORES)], axis=0)
    return out.reshape(B, N, E * 2 * Z).astype(np.float32)
